# revision 30
# baseline (speedup 1.0000x reference)
"""Trainium2 Bass kernel for nn_DMHA_3255585210402 (retrieval_knn DMHA).

Key algebraic fact: TOPK == NVK == 4, so jax.lax.top_k over the size-4 v_keys
axis selects *all* entries; the gather+sum over (DVH, TOPK) reduces to a
constant vector c = 2 * v_embed[0:4].sum(0) and compute_value_states collapses
to  v = x * c  (c is folded into the xg input host-side, so V is free).

The module is causal MHA (B=2, H=16, T=2048, HD=128, D=2048) with
elementwise-scaled V.  Sharding: 8 cores = 2 batches x 4 head-groups; each
core does qT/kT projections, attention for its 4 heads, and the partial
output projection Wo[:, gsl].T @ oT; host sums 4 partials per batch + bo.

Schedule (PE-bound, ~240us of matmul columns at 2.4GHz):
 - everything bf16 into the PE (1 cyc/row, same as f32r with 512-wide
   moving dim, but half the DMA/SBUF and 2x DVE tensor_tensor mode);
   PSUM and biases stay fp32, out partials are bf16 (summed fp32 on host).
 - DMA instruction ISSUE (~330-660ns each on the sync sequencer) bounds
   startup, so phase-A inputs load as few wide DMAs (dk-quarters of x and
   weights interleaved) and the tci=0 q-pass runs dk-outer with 4 psum
   accumulators so the PE consumes x quarters in arrival order.
 - small consts + a dummy Exp go first so the Act engine's table load and
   first waits bind to an early DMA tick (otherwise the scheduler ties
   them to much later input batches and phase A stalls on PSUM drain).
 - tci >= 1 runs per-(proj,head)-sequential over dk (~2 live PSUM banks,
   one shared pool, no pool-swap stall into phase B); B(0), the shortest
   Act-paced chunk, is emitted as filler units inside phase A's stream
   where Act/DVE/gpsimd are idle.
 - phase B per (j,h): QK emitted L=5 ahead of PV to cover the QK->exp->PV
   chain; diagonal causal masks via gpsimd affine_select; colsum = bf16
   DVE tensor_add pair/quad/acc tree + ONE ones-matmul at finalize.
 - softmax normalize (ones-matmul -> recip -> gpsimd row-broadcast ->
   DVE mult) is deferred one head block; out-projection dk-groups for
   chunk j-1 ride INSIDE chunk j's i-loops at spaced positions, filling
   the PE exactly where the Act engine's exp lag (687ns/exp vs ~480ns of
   QK+PV) would otherwise stall the pipeline; the (2,5,5,2|4) split keeps
   every C group after the normalize of the heads it reads.
"""

import math

import numpy as np
import ml_dtypes

import concourse.bass as bass
import concourse.mybir as mybir
import concourse.tile as tile
from concourse import bacc
from concourse.bass_utils import run_bass_kernel_spmd

B, T, D = 2, 2048, 2048
H, HD = 16, 128
G = 4              # head-groups (cores per batch)
GH = H // G        # heads per core
GF = GH * HD       # projected features per core (512)
NCORES = 8
P = 128            # partitions
TQ = 512           # tq chunk width (psum bank / fp32 moving max)
F32 = mybir.dt.float32
BF16 = mybir.dt.bfloat16

DK = D // P        # 16 contraction chunks for projections
NTQ = T // TQ      # 4 query chunks
NTK = T // P       # 16 key chunks



def _body(tc, xT, xg, wqh, wkh, woT, bqT, bkT, ones, out):
    nc = tc.nc
    rsqrt_hd = 1.0 / math.sqrt(HD)

    with (
        tc.tile_pool(name="const", bufs=1) as const,
        tc.tile_pool(name="res", bufs=1) as res,
        tc.tile_pool(name="psB", bufs=5, space="PSUM") as psB,
        tc.tile_pool(name="psO", bufs=2, space="PSUM") as psO,
        tc.tile_pool(name="psSum", bufs=1, space="PSUM") as psSum,
        tc.tile_pool(name="wt", bufs=11) as wtp,
        tc.tile_pool(name="quad", bufs=7) as qdp,
        tc.tile_pool(name="rb", bufs=2) as rbp,
        tc.tile_pool(name="recip", bufs=2) as rcp,
        tc.tile_pool(name="ct", bufs=4) as ctp,
    ):
        qT_sb = res.tile([P, GH, T], BF16)    # q, transposed per head
        kT_sb = res.tile([P, GH, T], BF16)
        oT_sb = res.tile([P, GH, T], BF16)    # attention out (normalized)
        wq_sb = res.tile([P, GH, DK * HD], BF16)
        wk_sb = res.tile([P, GH, DK * HD], BF16)
        xg_sb = res.tile([P, NTK, GF], BF16)  # c-scaled x[:, gsl] by tk chunk
        wo_sb = res.tile([P, GH, D], BF16)    # Wo[:, gsl].T chunked by head
        ones_sb = const.tile([P, 1], BF16)
        bq_sb = const.tile([HD, GH], F32)
        bk_sb = const.tile([HD, GH], F32)

        wor = woT.rearrange("(m p) d -> p m d", p=P)

        # ---- shared B-phase emission helpers ----
        state = [None]   # previous head block awaiting finalization
        cfill = []       # (j, dk) out-proj groups awaiting emission

        def emit_c_group(cj, cdk):
            qsl = slice(cj * TQ, (cj + 1) * TQ)
            ps = psB.tile([P, TQ], F32, name="psC", tag="psb")
            for m in range(GH):
                nc.tensor.matmul(
                    ps,
                    wo_sb[:, m, cdk * P : (cdk + 1) * P],
                    oT_sb[:, m, qsl],
                    start=(m == 0),
                    stop=(m == GH - 1),
                )
            ct = ctp.tile([P, TQ], BF16, name="ct")
            nc.vector.tensor_copy(ct, ps)
            nc.sync.dma_start(out=out[cdk * P : (cdk + 1) * P, qsl], in_=ct)

        def finalize(st):
            # column sum: one ones-matmul over the DVE-merged qacc, then
            # recip -> gpsimd row-broadcast -> DVE normalize
            qsl = slice(st["j"] * TQ, (st["j"] + 1) * TQ)
            ps_sum = psSum.tile([1, TQ], F32, name="ps_sum")
            nc.tensor.matmul(ps_sum, ones_sb, st["qacc"], start=True, stop=True)
            recip = rcp.tile([1, TQ], F32, name="recip")
            nc.vector.reciprocal_approx_fast(out=recip, in_=ps_sum)
            rb = rbp.tile([P, TQ], F32, name="rb")
            nc.gpsimd.partition_broadcast(rb, recip)
            nc.vector.tensor_mul(oT_sb[:, st["h"], qsl], st["ps_o"], rb)

        def emit_qk(st, idx):
            j, h, nkk = st["j"], st["h"], st["nkk"]
            qsl = slice(j * TQ, (j + 1) * TQ)
            ps_s = psB.tile([P, TQ], F32, name="ps_s", tag="psb")
            nc.tensor.matmul(
                ps_s,
                kT_sb[:, h, idx * P : (idx + 1) * P],
                qT_sb[:, h, qsl],
                start=True,
                stop=True,
            )
            wt = wtp.tile([P, TQ], BF16, name="wt")
            nc.scalar.activation(
                wt, ps_s, mybir.ActivationFunctionType.Exp, scale=rsqrt_hd,
            )
            g = idx - (TQ // P) * j
            if g >= 0:  # diagonal tile: zero where tk > tq
                nc.gpsimd.affine_select(
                    out=wt, in_=wt, pattern=[[1, TQ]],
                    compare_op=mybir.AluOpType.is_ge,
                    fill=0.0, base=-(P * g), channel_multiplier=-1,
                )
            st["wts"][idx] = wt
            if idx % 2 == 1:
                wp = wtp.tile([P, TQ], BF16, name="wp")
                nc.vector.tensor_add(wp, st["wts"][idx - 1], wt)
                if idx % 4 == 3:
                    wq_t = qdp.tile([P, TQ], BF16, name="wq4")
                    nc.vector.tensor_add(wq_t, st["wpair"], wp)
                    if st["qacc"] is None:
                        st["qacc"] = wq_t
                    else:
                        nc.vector.tensor_add(st["qacc"], st["qacc"], wq_t)
                st["wpair"] = wp

        def emit_pv(st, i):
            nc.tensor.matmul(
                st["ps_o"],
                xg_sb[:, i, st["h"] * HD : (st["h"] + 1) * HD],
                st["wts"][i],
                start=(i == 0),
                stop=(i == st["nkk"] - 1),
            )

        def new_state(j, h, nkk):
            return {
                "h": h, "j": j, "nkk": nkk, "qacc": None,
                "wts": [None] * nkk, "wpair": None,
                "ps_o": psO.tile([P, TQ], F32, name="ps_o"),
            }

        def gen_b0():
            """B(0) head blocks as filler units for the phase-A stream."""
            for h in range(GH):
                st = new_state(0, h, TQ // P)
                for i in range(4):
                    emit_qk(st, i)
                    yield
                for i in range(4):
                    emit_pv(st, i)
                    yield
                if state[0] is not None:
                    finalize(state[0])
                state[0] = st
                yield
            finalize(state[0])
            state[0] = None
            cfill.extend((0, dk) for dk in range(DK))
            while True:
                yield

        # --- phase A: q/k projections + embedded B(0) ---
        with tc.tile_pool(name="xt", bufs=8) as xtp:
            # Small constants first so the Act engine's first waits (and its
            # table load) bind to an early DMA tick, then weights/x.
            nc.sync.dma_start(out=ones_sb, in_=ones)
            nc.sync.dma_start(out=bq_sb, in_=bqT)
            nc.sync.dma_start(out=bk_sb, in_=bkT)
            # dummy activation: forces the act table load + first Act
            # instruction to depend only on an immediately-ready scratch
            dmy = const.tile([1, 1], F32)
            nc.vector.memset(dmy, 0.0)
            nc.scalar.activation(dmy, dmy, mybir.ActivationFunctionType.Exp)

            xTr = xT.rearrange("(n p) t -> p n t", p=P)
            xgr = xg.rearrange("(n p) f -> p n f", p=P)
            wqr2 = wqh.rearrange("(h p) f -> p h f", h=GH)
            wkr2 = wkh.rearrange("(h p) f -> p h f", h=GH)

            def load_xq(pool, qi, tsl):
                """One dk-quarter of x for a tq chunk: single wide DMA."""
                xq = pool.tile([P, 4, TQ], BF16, name="xq")
                nc.sync.dma_start(out=xq, in_=xTr[:, qi * 4 : (qi + 1) * 4, tsl])
                return xq

            # tci=0 is issue-latency critical: interleave dk-quarters of
            # wq with x quarters so the dk-outer q-pass can start ~2 wide
            # DMAs in and stream; wk/xg follow for the k-pass and B(0).
            xts = [None] * 4
            for qi in range(4):
                csl = slice(qi * GH * HD, (qi + 1) * GH * HD)
                nc.sync.dma_start(out=wq_sb[:, :, csl], in_=wqr2[:, :, csl])
                xts[qi] = load_xq(xtp, qi, slice(0, TQ))
            for qi in range(4):
                csl = slice(qi * GH * HD, (qi + 1) * GH * HD)
                nc.sync.dma_start(out=wk_sb[:, :, csl], in_=wkr2[:, :, csl])
            # embedded B(0) PV inputs (tk chunks 0..3), one wide DMA
            nc.sync.dma_start(out=xg_sb[:, 0:4, :], in_=xgr[:, 0:4, :])
            from concourse import library_config
            nc.gpsimd.load_library(library_config.attn)

            b0 = gen_b0()
            for tci in range(NTQ):
                tsl = slice(tci * TQ, (tci + 1) * TQ)
                if tci + 1 < NTQ:
                    nsl = slice((tci + 1) * TQ, (tci + 2) * TQ)
                    nxts = [None] * 4
                if tci == 2:
                    nc.sync.dma_start(out=xg_sb[:, 4:NTK, :], in_=xgr[:, 4:NTK, :])
                    for m in range(GH):
                        nc.sync.dma_start(out=wo_sb[:, m, :], in_=wor[:, m, :])
                if tci == 0:
                    # q-pass dk-outer: 4 accumulators consume x tiles in
                    # arrival order so the PE rides the initial x DMA
                    psq = [
                        psB.tile([P, TQ], F32, name="psA", tag="psb")
                        for _ in range(GH)
                    ]
                    for dk in range(DK):
                        for h in range(GH):
                            nc.tensor.matmul(
                                psq[h],
                                wq_sb[:, h, dk * HD : (dk + 1) * HD],
                                xts[dk // 4][:, dk % 4, :],
                                start=(dk == 0),
                                stop=(dk == DK - 1),
                            )
                    for h in range(GH):
                        nc.scalar.activation(
                            qT_sb[:, h, tsl],
                            psq[h],
                            mybir.ActivationFunctionType.Identity,
                            bias=bq_sb[:, h : h + 1],
                        )
                    groups = [(wk_sb, kT_sb, bk_sb, h) for h in range(GH)]
                else:
                    groups = [(wq_sb, qT_sb, bq_sb, h) for h in range(GH)]
                    groups += [(wk_sb, kT_sb, bk_sb, h) for h in range(GH)]
                for gi, (w_sb, dstT, bias, h) in enumerate(groups):
                    if tci + 1 < NTQ and gi % (len(groups) // 4) == 0:
                        qi = gi // (len(groups) // 4)
                        nxts[qi] = load_xq(xtp, qi, nsl)
                    ps = psB.tile([P, TQ], F32, name="psA", tag="psb")
                    for dk in range(DK):
                        nc.tensor.matmul(
                            ps,
                            w_sb[:, h, dk * HD : (dk + 1) * HD],
                            xts[dk // 4][:, dk % 4, :],
                            start=(dk == 0),
                            stop=(dk == DK - 1),
                        )
                        if tci >= 1 and dk % 8 == 7:
                            next(b0)
                    nc.scalar.activation(
                        dstT[:, h, tsl],
                        ps,
                        mybir.ActivationFunctionType.Identity,
                        bias=bias[:, h : h + 1],
                    )
                if tci + 1 < NTQ:
                    xts = nxts
            # drain any remaining B(0) units
            for _ in range(48):
                next(b0)

        # --- phases B+C: chunks j=1..3 + interleaved out-projection ---
        # C groups ride INSIDE the i-loop at spaced positions, where the
        # Act engine's exp lag (687ns/exp vs ~480ns of QK+PV) accrues.
        CPOS = (3, 6, 9, 12, 15, 18)
        for j in range(1, NTQ):
            nkk = (j + 1) * (TQ // P)
            L = 5
            for h in range(GH):
                share = (2, 5, 5, 4)[h] if j < NTQ - 1 else (2, 5, 5, 2)[h]
                n_c = min(share, len(cfill))
                mine = cfill[:n_c]
                del cfill[:n_c]
                # at h==0 the C groups are same-chunk as the pending
                # normalize, so finalize must precede them.
                if h == 0 and state[0] is not None:
                    finalize(state[0])
                    state[0] = None
                n_pos = sum(1 for p in CPOS if p < nkk + L)
                for (cj, cdk) in mine[n_pos:]:  # overflow at flush
                    emit_c_group(cj, cdk)
                mine = mine[:n_pos]
                if state[0] is not None:
                    finalize(state[0])
                st = new_state(j, h, nkk)
                ci = 0
                for idx in range(nkk + L):
                    if idx < nkk:
                        emit_qk(st, idx)
                    if idx >= L:
                        emit_pv(st, idx - L)
                    if idx in CPOS and ci < len(mine):
                        emit_c_group(*mine[ci])
                        ci += 1
                state[0] = st
                if h == GH - 1:
                    cfill.extend((j, dk) for dk in range(DK))
        # tail: two held-back chunk-2 groups fill the final finalize's
        # quad-chain wait, then C(3)
        for (cj, cdk) in cfill[:2]:
            emit_c_group(cj, cdk)
        del cfill[:2]
        finalize(state[0])
        for (cj, cdk) in cfill:
            emit_c_group(cj, cdk)


def build_program():
    nc = bacc.Bacc(
        "TRN2", target_bir_lowering=False, debug=False, num_devices=NCORES
    )
    xT = nc.dram_tensor("xT", [D, T], BF16, kind="ExternalInput").ap()
    xg = nc.dram_tensor("xg", [T, GF], BF16, kind="ExternalInput").ap()
    wqh = nc.dram_tensor("wqh", [GH * P, DK * HD], BF16, kind="ExternalInput").ap()
    wkh = nc.dram_tensor("wkh", [GH * P, DK * HD], BF16, kind="ExternalInput").ap()
    woT = nc.dram_tensor("woT", [GF, D], BF16, kind="ExternalInput").ap()
    bqT = nc.dram_tensor("bqT", [HD, GH], F32, kind="ExternalInput").ap()
    bkT = nc.dram_tensor("bkT", [HD, GH], F32, kind="ExternalInput").ap()
    ones = nc.dram_tensor("ones", [P, 1], BF16, kind="ExternalInput").ap()
    out = nc.dram_tensor("out", [D, T], BF16, kind="ExternalOutput").ap()

    with tile.TileContext(nc) as tc:
        _body(tc, xT, xg, wqh, wkh, woT, bqT, bkT, ones, out)
    nc.compile()
    return nc


def _whead(W: np.ndarray) -> np.ndarray:
    """[GF, D] weight slice -> per-head DMA layout [GH*P, DK*HD]."""
    return np.ascontiguousarray(
        W.reshape(GH, HD, DK, P).transpose(0, 3, 2, 1).reshape(GH * P, DK * HD)
    )


_NC_CACHE = None
LAST_RESULT = None
TRACE = False


def kernel(x, Wq, bq, Wk, bk, Wvq, bvq, v_keys, v_embed, Wo, bo):
    global _NC_CACHE, LAST_RESULT
    bf = ml_dtypes.bfloat16
    x = np.asarray(x, np.float32)
    Wq = np.asarray(Wq, np.float32)
    bq = np.asarray(bq, np.float32)
    Wk = np.asarray(Wk, np.float32)
    bk = np.asarray(bk, np.float32)
    v_embed = np.asarray(v_embed, np.float32)
    Wo = np.asarray(Wo, np.float32)
    bo = np.asarray(bo, np.float32)

    c = 2.0 * v_embed[:G].sum(axis=0)
    in_maps = []
    for core in range(NCORES):
        b, g = divmod(core, G)
        gsl = slice(g * GF, (g + 1) * GF)
        in_maps.append(
            {
                "xT": np.ascontiguousarray(x[b].T).astype(bf),
                "xg": np.ascontiguousarray(x[b][:, gsl] * c[gsl]).astype(bf),
                "wqh": _whead(Wq[gsl, :]).astype(bf),
                "wkh": _whead(Wk[gsl, :]).astype(bf),
                "woT": np.ascontiguousarray(Wo[:, gsl].T).astype(bf),
                "ones": np.ones((P, 1), bf),
                "bqT": np.ascontiguousarray(bq[gsl].reshape(GH, HD).T),
                "bkT": np.ascontiguousarray(bk[gsl].reshape(GH, HD).T),
            }
        )

    if _NC_CACHE is None:
        _NC_CACHE = build_program()
    res = run_bass_kernel_spmd(
        _NC_CACHE, in_maps, list(range(NCORES)), trace=TRACE
    )
    LAST_RESULT = res

    out = np.zeros((B, T, D), np.float32)
    for core in range(NCORES):
        b = core // G
        out[b] += res.results[core]["out"].T.astype(np.float32)
    out += bo[None, None, :]
    return out


if __name__ == "__main__":
    nc = build_program()
    print("built ok")
